# revision 5
# baseline (speedup 1.0000x reference)
# Trainium2 Bass kernel for nn_AttentionModule_70136815943908.
#
# Reference computation (per batch b, with xf = x[b] viewed [C, N], N = H*W):
#   qk = w1 @ xf + b1                       [D, N]
#   v  = w2 @ xf + b2                       [C, N]
#   S  = qk^T @ qk                          [N, N]   (symmetric Gram matrix)
#   A  = softmax(S, axis=-1)
#   O  = A @ v^T                            [N, C]
#   y  = x + O.flat-viewed-as-[C, H, W]     (reference reshapes [N,C] -> [C,H,W])
#
# Key structure (see work/kernel_v1_baseline.py for the f32r ancestor):
#   * S symmetric => E = exp(S - c) with a GLOBAL shift c is symmetric, so
#     stored E tiles serve directly as the pre-transposed lhsT of the P
#     matmul.  rowsum(E) comes free via the ACT accum_out; the global shift
#     is safe (diag(S) >= 0 bounds row maxes; measured |S| <= ~152, so
#     E <= e^64 -- needs bf16/fp32 exponent range, NOT fp16).
#   * v^T computed directly as x^T @ w2^T with host-pretransposed w2.
#   * The quirky output reshape is a pure flat add: flat(y) = flat(x)+flat(O).
#   * b2 added post-attention (softmax rows sum to 1), fused into the
#     prefetched fp32 x-residual tiles.
#
# bf16 conversion (this version; ~11% over the f32r kernel in-window):
#   * ALL matmul operands bf16 (x, w1t, w2t, qk, E, vt), PSUM fp32.
#     f32r InstMatmult self-loads weights serially (~+35ns/MM); bf16 gets
#     a separate LDWEIGHTS the PE pulls ahead into the background weight
#     buffer + FWL.  HW-measured (For_i differential, rotating lhsT,
#     512-col moving): f32r 249 ns/MM vs bf16 237 ns/MM.
#   * K=64 matmuls UNPACKED are pathological: ~500-525 ns/MM (f32r and
#     bf16 alike).  The S phase (contraction = D = 64) is therefore
#     row-packed: qk is stored row-duplicated [128, N] and S MM pairs
#     (m, m+1) run concurrently on row groups 0:64 / 64:128 via
#     tile_position auto-derivation -> 363 ns per pair.
#   * qk row-duplication comes free: w1t is host-duplicated to [C, 128]
#     so the qk matmul emits [128, 512] (full M) in one MM -- measured
#     faster (237 ns) than M=64 col-packed pairs (384 ns).
#   * Accuracy: bf16 operand quantization gives rel err 3.6e-3 vs the
#     fp32 reference (tolerance 2e-2; numpy-simulated 3.85e-3).  The
#     residual path also reads the bf16 x upload (single upload; was
#     fp32 double-upload) -- costs only +0.07e-3 rel err and saves
#     1 MB/batch of DMA plus half the DVE stt input-read volume:
#     measured -2.4us in-window (118.8 vs 121.1 us).  fp8 is DEAD:
#     single-fp8 v measured 3.1e-2 (fails), hi+lo 2-pass ties f32r.
#   * 1024-col bf16 moving operands CRASH walrus codegen (lower/codegen
#     pass) -- do not retry.
#   * Interleaving P/v psum bank pairs (p64alt) measured ~3ns/MM in
#     isolation but was a wash in situ -- dropped.
#   * Measured in-window A/B (device-resident For_i differential,
#     r=512/1024): f32r baseline 133.1us -> this kernel 118.9us
#     (~29.7-30.1 us/batch).  Static-operand floor of the exact MM
#     sequence (kseq2): 26.3 us/batch -> ~87% overlap efficiency.
#   * Sharding: data-parallel over batch, 4 batches per core on 8 cores,
#     weights replicated, no cross-core communication.
#
# Scheduling dead ends (HW A/B-measured in-window, do not repeat):
#   * Batched reciprocal (1 DVE op for 8 rowsums) + residual loads on the
#     gpsimd DMA queue: +4us.  Paired 2-bank evacuations (v pairs / qk as
#     single [128,1024] DVE ops, 3x2-bank PSUM pools): +9us.  Split-half
#     exps ([128,512] x2 + partial-accum adds): +2us.  Lesson: small
#     DISTRIBUTED consumer ops that free PSUM banks early beat batched
#     ops; the ~30ns/MM consumer-coupling gap (kseq2 static floor 26.1
#     vs kdve 29.7 us/batch) is per-instruction queue/wait overhead, not
#     sync-point count, and none of these restructurings reduce it.
#   * Gap split (kdve_nodve / kdve_noact): the ACT exp's PSUM reads are
#     ~2/3 of the coupling gap (~2.9us/batch), DVE evacs ~1/3 (~1.2).
#     Likely PSUM port arbitration vs PE writes: S tiles (banks 0-3) and
#     v/P tiles (banks 4-7) are already in separate bank groups, exp
#     volume is fixed, and splitting exps in half makes it worse -- no
#     software lever found.  gpsimd cannot take elementwise evac ops
#     (walrus lowering crash).
#
import sys

for _p in ("/opt/trn_rl_repo", "/opt/pypackages"):
    if _p not in sys.path:
        sys.path.insert(0, _p)

import numpy as np
import ml_dtypes

import concourse.bass as bass
import concourse.tile as tile
from concourse import bacc, mybir
from concourse.bass_utils import run_bass_kernel_spmd

B, C, H, W = 32, 512, 32, 32
N = H * W          # 1024
D = C // 8         # 64
NCORES = 8
NB = B // NCORES   # batches per core
KC = C // 128      # 4 contraction chunks over channels
NBLK = N // 128    # 8 blocks over sequence
C_SHIFT = 88.0     # global softmax shift
FP = mybir.dt.float32
BF = mybir.dt.bfloat16

_program_cache = {}
LAST_RESULTS = None


def _build_program(repeat=None, nb=None):
    nc = bacc.Bacc("TRN2", target_bir_lowering=False, debug=False)

    nb = NB if nb is None else nb
    xb_d = nc.dram_tensor("xb", [nb, C, N], BF, kind="ExternalInput")
    w1t_d = nc.dram_tensor("w1t", [C, 2 * D], BF, kind="ExternalInput")
    b1_d = nc.dram_tensor("b1", [2 * D, 1], FP, kind="ExternalInput")
    w2t_d = nc.dram_tensor("w2t", [C, C], BF, kind="ExternalInput")
    b2_d = nc.dram_tensor("b2", [1, C], FP, kind="ExternalInput")
    y_d = nc.dram_tensor("y", [nb, C, N], FP, kind="ExternalOutput")

    import contextlib

    with tile.TileContext(nc) as tc:
        with (
            tc.tile_pool(name="consts", bufs=1) as consts,
            tc.tile_pool(name="xin", bufs=2) as xpool,
            tc.tile_pool(name="qk", bufs=2) as qkpool,
            tc.tile_pool(name="vt", bufs=2) as vpool,
            tc.tile_pool(name="ee", bufs=2) as epool,
            tc.tile_pool(name="rr", bufs=2) as rpool,
            tc.tile_pool(name="oo", bufs=3) as opool,
            tc.tile_pool(name="xa", bufs=2) as xapool,
            tc.tile_pool(name="ps_s", bufs=2, space="PSUM") as ps_s,
            tc.tile_pool(name="ps_vp", bufs=4, space="PSUM") as ps_vp,
        ):
            # ---- replicated constants ----
            w1t_sb = consts.tile([128, KC, 2 * D], BF)
            nc.sync.dma_start(
                w1t_sb[:], w1t_d.ap().rearrange("(j p) d -> p j d", p=128)
            )
            b1_sb = consts.tile([2 * D, 1], FP)
            nc.sync.dma_start(b1_sb[:], b1_d.ap())
            w2t_sb = consts.tile([128, KC, C], BF)
            b2b_sb = consts.tile([128, C], FP)
            negc_sb = consts.tile([128, 1], FP)
            nc.vector.memset(negc_sb[:], -C_SHIFT)
            nc.scalar.dma_start(
                w2t_sb[:],
                w2t_d.ap().rearrange("(j p) o -> p j o", p=128),
            )
            nc.scalar.dma_start(
                b2b_sb[:], bass.AP(tensor=b2_d, offset=0, ap=[[0, 128], [1, C]])
            )
            # PE p-state warm-up (~4us of fp32 matmuls, no DMA dependency)
            warm_sb = consts.tile([128, 512], FP)
            nc.vector.memset(warm_sb[:], 0.0)
            warm_ps = ps_vp.tile([1, 512], FP, tag="vp")
            for _w in range(6):
                nc.tensor.matmul(
                    warm_ps[:], lhsT=warm_sb[:, 0:1], rhs=warm_sb[:],
                    start=True, stop=True,
                )

            loop_cm = tc.For_i(0, repeat, 1) if repeat else contextlib.nullcontext()
            with loop_cm:
                _emit_body(nc, tc, locals(), nb)
    nc.compile()
    return nc


def _emit_body(nc, tc, env, nb=NB):
    xb_d, y_d = env["xb_d"], env["y_d"]
    w1t_sb, b1_sb, w2t_sb, b2b_sb, negc_sb = (
        env["w1t_sb"], env["b1_sb"], env["w2t_sb"], env["b2b_sb"], env["negc_sb"]
    )
    xpool, qkpool, vpool, epool, rpool, opool = (
        env["xpool"], env["qkpool"], env["vpool"], env["epool"], env["rpool"], env["opool"]
    )
    xapool = env["xapool"]
    ps_s, ps_vp = env["ps_s"], env["ps_vp"]

    def load_x(bq):
        xb = xb_d.ap()[bq].rearrange("(j p) n -> p j n", p=128)
        xt = xpool.tile([128, KC, N], BF, name="x_sb")
        for j in range(KC):
            nc.sync.dma_start(xt[:, j, :], xb[:, j, :])
        return xt

    def emit_qk(x_sb):
        # qk = w1 @ x + b1 : computed twice via col packing -> [128, N] with
        # rows 0:64 and 64:128 both holding qk (row-duplicated for S packing).
        qk_sb = qkpool.tile([128, N], BF)
        for h in range(2):
            qk_ps = ps_vp.tile([128, 512], FP, tag="vp", name="qk_ps")
            for j in range(KC):
                nc.tensor.matmul(
                    qk_ps[:],
                    lhsT=w1t_sb[:, j, :],
                    rhs=x_sb[:, j, h * 512 : (h + 1) * 512],
                    start=(j == 0),
                    stop=(j == KC - 1),
                )
            nc.vector.tensor_scalar_add(
                qk_sb[:, h * 512 : (h + 1) * 512], qk_ps[:], b1_sb[:],
            )
        return qk_sb

    x_tiles = {}
    qk_next = None
    for bi in range(nb):
        x_sb = x_tiles.pop(bi) if bi in x_tiles else load_x(bi)

        qk_sb = qk_next if qk_next is not None else emit_qk(x_sb)
        qk_next = None

        # prefetch the flat-view bf16 x for the residual add
        xflat_pre = (
            xb_d.ap()[bi]
            .rearrange("c n -> (c n)")
            .rearrange("(i p f) -> p i f", p=128, f=C)
        )
        xb2_sb = xapool.tile([128, NBLK, C], BF, name="xb2")
        for i in range(NBLK):
            nc.sync.dma_start(xb2_sb[:, i, :], xflat_pre[:, i, :])

        if bi + 1 < nb:
            x_tiles[bi + 1] = load_x(bi + 1)

        # ---- interleaved: S pairs (row-packed) between vT m-groups ----
        e_sb = epool.tile([128, NBLK, N], BF)
        r_sb = rpool.tile([128, NBLK], FP, tag="rsum")
        rr_sb = rpool.tile([128, NBLK], FP, tag="rinv")
        vt_sb = vpool.tile([128, NBLK, C], BF)

        def emit_s_pair(ma, mb):
            sa = ps_s.tile([128, N], FP, tag="s")
            sb = ps_s.tile([128, N], FP, tag="s")
            for h in range(2):
                nc.tensor.matmul(
                    sa[:, h * 512 : (h + 1) * 512],
                    lhsT=qk_sb[0:64, ma * 128 : (ma + 1) * 128],
                    rhs=qk_sb[0:64, h * 512 : (h + 1) * 512],
                    start=True,
                    stop=True,
                )
                nc.tensor.matmul(
                    sb[:, h * 512 : (h + 1) * 512],
                    lhsT=qk_sb[64:128, mb * 128 : (mb + 1) * 128],
                    rhs=qk_sb[64:128, h * 512 : (h + 1) * 512],
                    start=True,
                    stop=True,
                )
            for m, sp in ((ma, sa), (mb, sb)):
                nc.scalar.activation(
                    e_sb[:, m, :],
                    sp[:],
                    mybir.ActivationFunctionType.Exp,
                    bias=negc_sb[:],
                    scale=1.0,
                    accum_out=r_sb[:, m : m + 1],
                )
                nc.vector.reciprocal(rr_sb[:, m : m + 1], r_sb[:, m : m + 1])

        def emit_v(m):
            v_ps = ps_vp.tile([128, C], FP, tag="vp")
            for j in range(KC):
                nc.tensor.matmul(
                    v_ps[:],
                    lhsT=x_sb[:, j, m * 128 : (m + 1) * 128],
                    rhs=w2t_sb[:, j, :],
                    start=(j == 0),
                    stop=(j == KC - 1),
                )
            nc.vector.tensor_add(vt_sb[:, m, :], v_ps[:], b2b_sb[:])

        for mp in range(4):
            emit_s_pair(2 * mp, 2 * mp + 1)
            emit_v(2 * mp)
            emit_v(2 * mp + 1)

        # next batch's qk before the P phase (its DVE evacuation hides
        # under the P matmuls)
        if bi + 1 < nb:
            qk_next = emit_qk(x_tiles[bi + 1])

        # ---- P = E @ vT ; y.flat = P*rr + (x.flat + b2) ----
        yflat = (
            y_d.ap()[bi]
            .rearrange("c n -> (c n)")
            .rearrange("(i p f) -> p i f", p=128, f=C)
        )
        for i in range(NBLK):
            p_ps = ps_vp.tile([128, C], FP, tag="vp")
            for k in range(NBLK):
                nc.tensor.matmul(
                    p_ps[:],
                    lhsT=e_sb[:, k, i * 128 : (i + 1) * 128],
                    rhs=vt_sb[:, k, :],
                    start=(k == 0),
                    stop=(k == NBLK - 1),
                )
            y_sb = opool.tile([128, C], FP, tag="y")
            nc.vector.scalar_tensor_tensor(
                y_sb[:],
                p_ps[:],
                rr_sb[:, i : i + 1],
                xb2_sb[:, i, :],
                op0=mybir.AluOpType.mult,
                op1=mybir.AluOpType.add,
            )
            nc.scalar.dma_start(yflat[:, i, :], y_sb[:])


def _get_program(repeat=None, nb=None):
    key = ("nc", repeat, nb)
    if key not in _program_cache:
        _program_cache[key] = _build_program(repeat, nb)
    return _program_cache[key]


def make_core_inputs(x_shard, w1t, b1r, w2t, b2r):
    x_shard = np.ascontiguousarray(x_shard, dtype=np.float32)
    w1t2 = np.concatenate([w1t, w1t], axis=1)  # [C, 128]: qk row-duplicated
    b1d = np.concatenate([b1r, b1r], axis=0)   # [128, 1]
    return {
        "xb": np.ascontiguousarray(x_shard.astype(ml_dtypes.bfloat16)),
        "w1t": np.ascontiguousarray(w1t2.astype(ml_dtypes.bfloat16)),
        "b1": np.ascontiguousarray(b1d, dtype=np.float32),
        "w2t": np.ascontiguousarray(w2t.astype(ml_dtypes.bfloat16)),
        "b2": np.ascontiguousarray(b2r, dtype=np.float32),
    }


def kernel(x, w1, b1, w2, b2, trace=False, trace_cores=None):
    global LAST_RESULTS
    nc = _get_program()

    x = np.ascontiguousarray(np.asarray(x, dtype=np.float32).reshape(B, C, N))
    w1t = np.ascontiguousarray(np.asarray(w1, dtype=np.float32).T)
    b1r = np.ascontiguousarray(np.asarray(b1, dtype=np.float32).reshape(D, 1))
    w2t = np.ascontiguousarray(np.asarray(w2, dtype=np.float32).T)
    b2r = np.ascontiguousarray(np.asarray(b2, dtype=np.float32).reshape(1, C))

    in_maps = [
        make_core_inputs(x[c * NB : (c + 1) * NB], w1t, b1r, w2t, b2r)
        for c in range(NCORES)
    ]

    kwargs = {}
    if trace:
        kwargs["trace"] = True
        if trace_cores is not None:
            kwargs["trace_cores"] = trace_cores
    res = run_bass_kernel_spmd(nc, in_maps, core_ids=list(range(NCORES)), **kwargs)
    LAST_RESULTS = res

    y = np.concatenate([res.results[c]["y"] for c in range(NCORES)], axis=0)
    return np.ascontiguousarray(y.reshape(B, C, H, W).astype(np.float32))


# revision 6
# speedup vs baseline: 1.0250x; 1.0250x over previous
# Trainium2 Bass kernel for nn_AttentionModule_70136815943908.
#
# Reference computation (per batch b, with xf = x[b] viewed [C, N], N = H*W):
#   qk = w1 @ xf + b1                       [D, N]
#   v  = w2 @ xf + b2                       [C, N]
#   S  = qk^T @ qk                          [N, N]   (symmetric Gram matrix)
#   A  = softmax(S, axis=-1)
#   O  = A @ v^T                            [N, C]
#   y  = x + O.flat-viewed-as-[C, H, W]     (reference reshapes [N,C] -> [C,H,W])
#
# Key structure (see work/kernel_v1_baseline.py for the f32r ancestor):
#   * S symmetric => E = exp(S - c) with a GLOBAL shift c is symmetric, so
#     stored E tiles serve directly as the pre-transposed lhsT of the P
#     matmul.  rowsum(E) comes free via the ACT accum_out; the global shift
#     is safe (diag(S) >= 0 bounds row maxes; measured |S| <= ~152, so
#     E <= e^64 -- needs bf16/fp32 exponent range, NOT fp16).
#   * v^T computed directly as x^T @ w2^T with host-pretransposed w2.
#   * The quirky output reshape is a pure flat add: flat(y) = flat(x)+flat(O).
#   * b2 added post-attention (softmax rows sum to 1), fused into the
#     prefetched fp32 x-residual tiles.
#
# bf16 conversion (this version; ~11% over the f32r kernel in-window):
#   * ALL matmul operands bf16 (x, w1t, w2t, qk, E, vt), PSUM fp32.
#     f32r InstMatmult self-loads weights serially (~+35ns/MM); bf16 gets
#     a separate LDWEIGHTS the PE pulls ahead into the background weight
#     buffer + FWL.  HW-measured (For_i differential, rotating lhsT,
#     512-col moving): f32r 249 ns/MM vs bf16 237 ns/MM.
#   * K=64 matmuls UNPACKED are pathological: ~500-525 ns/MM (f32r and
#     bf16 alike).  The S phase (contraction = D = 64) is therefore
#     row-packed: qk is stored row-duplicated [128, N] and S MM pairs
#     (m, m+1) run concurrently on row groups 0:64 / 64:128 via
#     tile_position auto-derivation -> 363 ns per pair.
#   * qk row-duplication comes free: w1t is host-duplicated to [C, 128]
#     so the qk matmul emits [128, 512] (full M) in one MM -- measured
#     faster (237 ns) than M=64 col-packed pairs (384 ns).
#   * Accuracy: bf16 operand quantization gives rel err 3.6e-3 vs the
#     fp32 reference (tolerance 2e-2; numpy-simulated 3.85e-3).  The
#     residual path also reads the bf16 x upload (single upload; was
#     fp32 double-upload) -- costs only +0.07e-3 rel err and saves
#     1 MB/batch of DMA plus half the DVE stt input-read volume:
#     measured -2.4us in-window (118.8 vs 121.1 us).  fp8 is DEAD:
#     single-fp8 v measured 3.1e-2 (fails), hi+lo 2-pass ties f32r.
#   * 1024-col bf16 moving operands CRASH walrus codegen (lower/codegen
#     pass) -- do not retry.
#   * Interleaving P/v psum bank pairs (p64alt) measured ~3ns/MM in
#     isolation but was a wash in situ -- dropped.
#   * Measured in-window A/B (device-resident For_i differential,
#     r=512/1024): f32r baseline 133.1us -> this kernel 118.9us
#     (~29.7-30.1 us/batch).  Static-operand floor of the exact MM
#     sequence (kseq2): 26.3 us/batch -> ~87% overlap efficiency.
#   * Sharding: data-parallel over batch, 4 batches per core on 8 cores,
#     weights replicated, no cross-core communication.
#
# Scheduling dead ends (HW A/B-measured in-window, do not repeat):
#   * Batched reciprocal (1 DVE op for 8 rowsums) + residual loads on the
#     gpsimd DMA queue: +4us.  Paired 2-bank evacuations (v pairs / qk as
#     single [128,1024] DVE ops, 3x2-bank PSUM pools): +9us.  Split-half
#     exps ([128,512] x2 + partial-accum adds): +2us.  Lesson: small
#     DISTRIBUTED consumer ops that free PSUM banks early beat batched
#     ops; the ~30ns/MM consumer-coupling gap (kseq2 static floor 26.1
#     vs kdve 29.7 us/batch) is per-instruction queue/wait overhead, not
#     sync-point count, and none of these restructurings reduce it.
#   * Gap split (kdve_nodve / kdve_noact): the ACT exp's PSUM reads are
#     ~2/3 of the coupling gap (~2.9us/batch), DVE evacs ~1/3 (~1.2).
#     Likely PSUM port arbitration vs PE writes: S tiles (banks 0-3) and
#     v/P tiles (banks 4-7) are already in separate bank groups, exp
#     volume is fixed, and splitting exps in half makes it worse -- no
#     software lever found.  gpsimd cannot take elementwise evac ops
#     (walrus lowering crash).
#   * b2b_sb (the vt-evac bias) in bf16: +1.4us -- small broadcast DVE
#     operands are free in fp32; only the big per-element input reads
#     (the xb2 residual tiles) were worth narrowing.
#
import sys

for _p in ("/opt/trn_rl_repo", "/opt/pypackages"):
    if _p not in sys.path:
        sys.path.insert(0, _p)

import numpy as np
import ml_dtypes

import concourse.bass as bass
import concourse.tile as tile
from concourse import bacc, mybir
from concourse.bass_utils import run_bass_kernel_spmd

B, C, H, W = 32, 512, 32, 32
N = H * W          # 1024
D = C // 8         # 64
NCORES = 8
NB = B // NCORES   # batches per core
KC = C // 128      # 4 contraction chunks over channels
NBLK = N // 128    # 8 blocks over sequence
C_SHIFT = 88.0     # global softmax shift
FP = mybir.dt.float32
BF = mybir.dt.bfloat16

_program_cache = {}
LAST_RESULTS = None


def _build_program(repeat=None, nb=None):
    nc = bacc.Bacc("TRN2", target_bir_lowering=False, debug=False)

    nb = NB if nb is None else nb
    xb_d = nc.dram_tensor("xb", [nb, C, N], BF, kind="ExternalInput")
    w1t_d = nc.dram_tensor("w1t", [C, 2 * D], BF, kind="ExternalInput")
    b1_d = nc.dram_tensor("b1", [2 * D, 1], FP, kind="ExternalInput")
    w2t_d = nc.dram_tensor("w2t", [C, C], BF, kind="ExternalInput")
    b2_d = nc.dram_tensor("b2", [1, C], FP, kind="ExternalInput")
    y_d = nc.dram_tensor("y", [nb, C, N], FP, kind="ExternalOutput")

    import contextlib

    with tile.TileContext(nc) as tc:
        with (
            tc.tile_pool(name="consts", bufs=1) as consts,
            tc.tile_pool(name="xin", bufs=2) as xpool,
            tc.tile_pool(name="qk", bufs=2) as qkpool,
            tc.tile_pool(name="vt", bufs=2) as vpool,
            tc.tile_pool(name="ee", bufs=2) as epool,
            tc.tile_pool(name="rr", bufs=2) as rpool,
            tc.tile_pool(name="oo", bufs=3) as opool,
            tc.tile_pool(name="xa", bufs=2) as xapool,
            tc.tile_pool(name="ps_s", bufs=2, space="PSUM") as ps_s,
            tc.tile_pool(name="ps_vp", bufs=4, space="PSUM") as ps_vp,
        ):
            # ---- replicated constants ----
            w1t_sb = consts.tile([128, KC, 2 * D], BF)
            nc.sync.dma_start(
                w1t_sb[:], w1t_d.ap().rearrange("(j p) d -> p j d", p=128)
            )
            b1_sb = consts.tile([2 * D, 1], FP)
            nc.sync.dma_start(b1_sb[:], b1_d.ap())
            w2t_sb = consts.tile([128, KC, C], BF)
            b2b_sb = consts.tile([128, C], FP)
            negc_sb = consts.tile([128, 1], FP)
            nc.vector.memset(negc_sb[:], -C_SHIFT)
            nc.scalar.dma_start(
                w2t_sb[:],
                w2t_d.ap().rearrange("(j p) o -> p j o", p=128),
            )
            nc.scalar.dma_start(
                b2b_sb[:], bass.AP(tensor=b2_d, offset=0, ap=[[0, 128], [1, C]])
            )
            # PE p-state warm-up (~4us of fp32 matmuls, no DMA dependency)
            warm_sb = consts.tile([128, 512], FP)
            nc.vector.memset(warm_sb[:], 0.0)
            warm_ps = ps_vp.tile([1, 512], FP, tag="vp")
            for _w in range(6):
                nc.tensor.matmul(
                    warm_ps[:], lhsT=warm_sb[:, 0:1], rhs=warm_sb[:],
                    start=True, stop=True,
                )

            loop_cm = tc.For_i(0, repeat, 1) if repeat else contextlib.nullcontext()
            with loop_cm:
                _emit_body(nc, tc, locals(), nb)
    nc.compile()
    return nc


def _emit_body(nc, tc, env, nb=NB):
    xb_d, y_d = env["xb_d"], env["y_d"]
    w1t_sb, b1_sb, w2t_sb, b2b_sb, negc_sb = (
        env["w1t_sb"], env["b1_sb"], env["w2t_sb"], env["b2b_sb"], env["negc_sb"]
    )
    xpool, qkpool, vpool, epool, rpool, opool = (
        env["xpool"], env["qkpool"], env["vpool"], env["epool"], env["rpool"], env["opool"]
    )
    xapool = env["xapool"]
    ps_s, ps_vp = env["ps_s"], env["ps_vp"]

    def load_x(bq):
        xb = xb_d.ap()[bq].rearrange("(j p) n -> p j n", p=128)
        xt = xpool.tile([128, KC, N], BF, name="x_sb")
        for j in range(KC):
            nc.sync.dma_start(xt[:, j, :], xb[:, j, :])
        return xt

    def emit_qk(x_sb):
        # qk = w1 @ x + b1 : computed twice via col packing -> [128, N] with
        # rows 0:64 and 64:128 both holding qk (row-duplicated for S packing).
        qk_sb = qkpool.tile([128, N], BF)
        for h in range(2):
            qk_ps = ps_vp.tile([128, 512], FP, tag="vp", name="qk_ps")
            for j in range(KC):
                nc.tensor.matmul(
                    qk_ps[:],
                    lhsT=w1t_sb[:, j, :],
                    rhs=x_sb[:, j, h * 512 : (h + 1) * 512],
                    start=(j == 0),
                    stop=(j == KC - 1),
                )
            nc.vector.tensor_scalar_add(
                qk_sb[:, h * 512 : (h + 1) * 512], qk_ps[:], b1_sb[:],
            )
        return qk_sb

    x_tiles = {}
    qk_next = None
    for bi in range(nb):
        x_sb = x_tiles.pop(bi) if bi in x_tiles else load_x(bi)

        qk_sb = qk_next if qk_next is not None else emit_qk(x_sb)
        qk_next = None

        # prefetch the flat-view bf16 x for the residual add
        xflat_pre = (
            xb_d.ap()[bi]
            .rearrange("c n -> (c n)")
            .rearrange("(i p f) -> p i f", p=128, f=C)
        )
        xb2_sb = xapool.tile([128, NBLK, C], BF, name="xb2")
        for i in range(NBLK):
            nc.sync.dma_start(xb2_sb[:, i, :], xflat_pre[:, i, :])

        if bi + 1 < nb:
            x_tiles[bi + 1] = load_x(bi + 1)

        # ---- interleaved: S pairs (row-packed) between vT m-groups ----
        e_sb = epool.tile([128, NBLK, N], BF)
        r_sb = rpool.tile([128, NBLK], FP, tag="rsum")
        rr_sb = rpool.tile([128, NBLK], FP, tag="rinv")
        vt_sb = vpool.tile([128, NBLK, C], BF)

        def emit_s_pair(ma, mb):
            sa = ps_s.tile([128, N], FP, tag="s")
            sb = ps_s.tile([128, N], FP, tag="s")
            for h in range(2):
                nc.tensor.matmul(
                    sa[:, h * 512 : (h + 1) * 512],
                    lhsT=qk_sb[0:64, ma * 128 : (ma + 1) * 128],
                    rhs=qk_sb[0:64, h * 512 : (h + 1) * 512],
                    start=True,
                    stop=True,
                )
                nc.tensor.matmul(
                    sb[:, h * 512 : (h + 1) * 512],
                    lhsT=qk_sb[64:128, mb * 128 : (mb + 1) * 128],
                    rhs=qk_sb[64:128, h * 512 : (h + 1) * 512],
                    start=True,
                    stop=True,
                )
            for m, sp in ((ma, sa), (mb, sb)):
                nc.scalar.activation(
                    e_sb[:, m, :],
                    sp[:],
                    mybir.ActivationFunctionType.Exp,
                    bias=negc_sb[:],
                    scale=1.0,
                    accum_out=r_sb[:, m : m + 1],
                )
                nc.vector.reciprocal(rr_sb[:, m : m + 1], r_sb[:, m : m + 1])

        def emit_v(m):
            v_ps = ps_vp.tile([128, C], FP, tag="vp")
            for j in range(KC):
                nc.tensor.matmul(
                    v_ps[:],
                    lhsT=x_sb[:, j, m * 128 : (m + 1) * 128],
                    rhs=w2t_sb[:, j, :],
                    start=(j == 0),
                    stop=(j == KC - 1),
                )
            nc.vector.tensor_add(vt_sb[:, m, :], v_ps[:], b2b_sb[:])

        for mp in range(4):
            emit_s_pair(2 * mp, 2 * mp + 1)
            emit_v(2 * mp)
            emit_v(2 * mp + 1)

        # next batch's qk before the P phase (its DVE evacuation hides
        # under the P matmuls)
        if bi + 1 < nb:
            qk_next = emit_qk(x_tiles[bi + 1])

        # ---- P = E @ vT ; y.flat = P*rr + (x.flat + b2) ----
        yflat = (
            y_d.ap()[bi]
            .rearrange("c n -> (c n)")
            .rearrange("(i p f) -> p i f", p=128, f=C)
        )
        for i in range(NBLK):
            p_ps = ps_vp.tile([128, C], FP, tag="vp")
            for k in range(NBLK):
                nc.tensor.matmul(
                    p_ps[:],
                    lhsT=e_sb[:, k, i * 128 : (i + 1) * 128],
                    rhs=vt_sb[:, k, :],
                    start=(k == 0),
                    stop=(k == NBLK - 1),
                )
            y_sb = opool.tile([128, C], FP, tag="y")
            nc.vector.scalar_tensor_tensor(
                y_sb[:],
                p_ps[:],
                rr_sb[:, i : i + 1],
                xb2_sb[:, i, :],
                op0=mybir.AluOpType.mult,
                op1=mybir.AluOpType.add,
            )
            nc.scalar.dma_start(yflat[:, i, :], y_sb[:])


def _get_program(repeat=None, nb=None):
    key = ("nc", repeat, nb)
    if key not in _program_cache:
        _program_cache[key] = _build_program(repeat, nb)
    return _program_cache[key]


def make_core_inputs(x_shard, w1t, b1r, w2t, b2r):
    x_shard = np.ascontiguousarray(x_shard, dtype=np.float32)
    w1t2 = np.concatenate([w1t, w1t], axis=1)  # [C, 128]: qk row-duplicated
    b1d = np.concatenate([b1r, b1r], axis=0)   # [128, 1]
    return {
        "xb": np.ascontiguousarray(x_shard.astype(ml_dtypes.bfloat16)),
        "w1t": np.ascontiguousarray(w1t2.astype(ml_dtypes.bfloat16)),
        "b1": np.ascontiguousarray(b1d, dtype=np.float32),
        "w2t": np.ascontiguousarray(w2t.astype(ml_dtypes.bfloat16)),
        "b2": np.ascontiguousarray(b2r, dtype=np.float32),
    }


def kernel(x, w1, b1, w2, b2, trace=False, trace_cores=None):
    global LAST_RESULTS
    nc = _get_program()

    x = np.ascontiguousarray(np.asarray(x, dtype=np.float32).reshape(B, C, N))
    w1t = np.ascontiguousarray(np.asarray(w1, dtype=np.float32).T)
    b1r = np.ascontiguousarray(np.asarray(b1, dtype=np.float32).reshape(D, 1))
    w2t = np.ascontiguousarray(np.asarray(w2, dtype=np.float32).T)
    b2r = np.ascontiguousarray(np.asarray(b2, dtype=np.float32).reshape(1, C))

    in_maps = [
        make_core_inputs(x[c * NB : (c + 1) * NB], w1t, b1r, w2t, b2r)
        for c in range(NCORES)
    ]

    kwargs = {}
    if trace:
        kwargs["trace"] = True
        if trace_cores is not None:
            kwargs["trace_cores"] = trace_cores
    res = run_bass_kernel_spmd(nc, in_maps, core_ids=list(range(NCORES)), **kwargs)
    LAST_RESULTS = res

    y = np.concatenate([res.results[c]["y"] for c in range(NCORES)], axis=0)
    return np.ascontiguousarray(y.reshape(B, C, H, W).astype(np.float32))
